# revision 24
# baseline (speedup 1.0000x reference)
"""Trainium2 Bass kernel for a causal self-attention block.

Reference computation (fp32):
    qkv = x @ W_qkv + b_qkv ; q,k,v = split(qkv)
    scores = (q @ k.T + mask) / sqrt(hd)
    wts = exp(scores) / (sum(exp(scores)) + 1e-9)
    y = (wts @ v) @ W_out + b_out
    out = LayerNorm(x + y) * gamma + beta

Sharding: 8 cores = 4 batches x 2 query-shards; the query chunks are
interleaved between the two shards of a batch so causal block-skipping
is balanced.  The block pattern is derived from the actual mask on the
host, so non-causal masks degrade gracefully to the dense kernel.

v2 design (vs the spill-based v1): everything lives in SBUF in bf16 —
K^T / Q^T / V / attn^T / z are resident, there are no HBM spills at all.
P1 streams x^T tiles once and produces K and V from the same tiles
(fused loops), then Q from the resident x_q.  P2 is fully SBUF-resident
per head with software pipelining: attn@V runs one k-tile behind the
exp so the PE never waits on the activation engine.  The softmax
denominator rides along as a ones column in V; exp covers only the
exact needed position prefix per k-tile (stale columns are zeroed by
the mask multiply; the num ring is pre-zeroed once so stale data is
always finite).  Masks, biases and the V ones-column are loop invariant
and hoisted out of the For_i timing loop; weights are double-buffered
across iterations via shared tags (K+Q, V+Out).  DMA issue is split
between the SP and Activation hardware queues.
"""

import numpy as np
import ml_dtypes

import concourse.bass as bass
import concourse.tile as tile
from concourse import bacc, mybir
from concourse.bass_utils import run_bass_kernel_spmd

# Problem dims (hardcoded per harness contract)
B, S, D, H = 4, 2048, 1024, 16
N_CORES = 8
QCHUNK = 256                     # q columns per position slot
KTILE = 128                      # k rows per tile
NEG_THRESH = -1.0e8              # mask <= this  =>  exp((qk+m)/8) == 0 in fp32
SM_EPS = 1.0e-9
LN_EPS = 1.0e-5

F32 = mybir.dt.float32
BF16 = mybir.dt.bfloat16

ALU = mybir.AluOpType
ACTF = mybir.ActivationFunctionType


# ----------------------------------------------------------------------------
# Host-side mask analysis / shard assignment
# ----------------------------------------------------------------------------

def _mask_pattern(attn_mask, b_, s_):
    """Classify mask blocks and build a core-uniform block schedule.

    Returns (chunk_at[b, shard, pos], kt_lists[pos] -> list of ktiles,
    tt_blocks -> ordered (pos, ktile) needing a mask multiply,
    exp_runs[t] -> (p0, p1) position-slot range the exp must cover).
    """
    nq = s_ // QCHUNK
    nk = s_ // KTILE
    m = attn_mask.reshape(b_, nq, QCHUNK, nk, KTILE)
    bmax = m.max(axis=(2, 4))
    bmin = m.min(axis=(2, 4))
    skip = bmax <= NEG_THRESH                    # contributes exactly 0
    zero = (bmin == 0.0) & (bmax == 0.0)         # no mask add needed
    needed = ~skip                               # [b, nq, nk]
    counts = needed.sum(axis=2)                  # [b, nq]

    order = np.argsort(-counts, axis=1, kind="stable")   # chunks by desc count
    npos = nq // 2
    chunk_at = np.zeros((b_, 2, npos), np.int64)
    chunk_at[:, 0, :] = order[:, 0::2]
    chunk_at[:, 1, :] = order[:, 1::2]

    kt_lists = []
    tt_blocks = []
    un_list = []
    tt_list = []
    for p in range(npos):
        un = np.zeros(nk, bool)
        for bb in range(b_):
            for sh in range(2):
                un |= needed[bb, chunk_at[bb, sh, p]]
        kt_lists.append(np.nonzero(un)[0].tolist())
        un_list.append(un)
        tt_list.append(np.zeros(nk, bool))
    # PSUM accumulation groups are per 512-col bank: fuse position pairs into
    # one group by unioning their k-tile lists; the extra blocks are masked.
    for j in range(0, npos - 1, 2):
        un2 = un_list[j] | un_list[j + 1]
        kt_lists[j] = kt_lists[j + 1] = np.nonzero(un2)[0].tolist()
        for p in (j, j + 1):
            # mask multiply needed wherever a core computes the block but its
            # mask is not identically zero there (incl. dead padding)
            any_tt = np.zeros(nk, bool)
            for bb in range(b_):
                for sh in range(2):
                    c = chunk_at[bb, sh, p]
                    any_tt |= un2 & ~(needed[bb, c] & zero[bb, c])
            tt_list[p] = any_tt
    for p in range(npos):
        for t in kt_lists[p]:
            if tt_list[p][t]:
                tt_blocks.append((p, t))

    # exp coverage per k-tile: the position-slot run [p0, p1) the exp reads.
    # Exact (only slots that truly need t) when those slots are contiguous;
    # otherwise the full pair-computed range.  Consumed-but-uncovered blocks
    # hold stale-but-finite data and are zeroed by the mask multiply (they
    # are in tt by construction).
    exp_runs = {}
    all_t = sorted({t for kl in kt_lists for t in kl})
    for t in all_t:
        ps = [p for p in range(npos) if un_list[p][t]]
        if ps and ps == list(range(ps[0], ps[-1] + 1)):
            exp_runs[t] = (ps[0], ps[-1] + 1)
        else:
            pr = [p for p in range(npos) if t in kt_lists[p]]
            exp_runs[t] = (min(pr), max(pr) + 1)
    return chunk_at, kt_lists, tt_blocks, exp_runs


# ----------------------------------------------------------------------------
# Device program
# ----------------------------------------------------------------------------

def _build_program(dims, kt_lists, tt_blocks, exp_runs,
                   n_iters=1, timing_mode=False,
                   phases=("p1", "p2", "p34")):
    """Emit the per-core Bass/Tile program (identical for all cores)."""
    b_, s_, d_, h_ = dims
    hd = d_ // h_                 # 64
    he = hd + 1                   # head dim + ones column
    sq = s_ // 2                  # queries per core
    npos = sq // QCHUNK           # 4
    nf = d_ // 128                # feature tiles
    nst = s_ // 128               # s tiles (token-major V) == n ktiles
    ndc = d_ // 128               # contraction d tiles
    nqr = s_ // 512               # x streaming quarters
    ntt = max(1, len(tt_blocks))
    tt_idx = {pt: i for i, pt in enumerate(tt_blocks)}
    kt_sets = [set(k) for k in kt_lists]
    pair_js = [j for j in range(0, npos, 2)]
    all_kt = sorted(set().union(*kt_sets)) if kt_lists else []
    inv_sqrt_hd = 1.0 / float(np.sqrt(hd))

    nc = bacc.Bacc("TRN2", target_bir_lowering=False, debug=False,
                   num_devices=N_CORES)

    # I/O.  In timing_mode the big inputs are internal DRAM scratch
    # (uninitialised) so per-call host->device transfer stays tiny.
    big = "Internal" if timing_mode else "ExternalInput"
    xT = nc.dram_tensor("xT", [d_, s_], BF16, kind=big).ap()
    xqT = nc.dram_tensor("xqT", [d_, sq], BF16, kind=big).ap()
    wqkv = nc.dram_tensor("wqkv", [d_, 3 * d_], BF16, kind=big).ap()
    wout = nc.dram_tensor("wout", [d_, d_], BF16, kind=big).ap()
    bpack = nc.dram_tensor("bpack", [128, 5 * nf], F32,
                           kind="ExternalInput").ap()
    bvrep = nc.dram_tensor("bvrep", [128, d_], BF16, kind="ExternalInput").ap()
    maskT = nc.dram_tensor("maskT", [ntt, KTILE, QCHUNK], BF16,
                           kind="ExternalInput").ap()
    onesb = nc.dram_tensor("onesb", [128, 128], BF16,
                           kind="ExternalInput").ap()
    yT = nc.dram_tensor("yT", [d_, sq], BF16, kind="ExternalOutput").ap()

    do_p1 = "p1" in phases
    do_p2 = "p2" in phases
    do_p34 = "p34" in phases

    def emit_consts(tc, P):
        """Loop-invariant data: biases, masks, ones, V ones-column."""
        U = P["u"]
        C = {}
        bpt = U.tile([128, 5 * nf], F32, name="bpt", tag="bpt", bufs=1)
        nc.scalar.dma_start(bpt[:], bpack[:])
        for i, nm in enumerate(("bq", "bk", "bo", "ga", "be")):
            C[nm] = [bpt[:, i * nf + f:i * nf + f + 1] for f in range(nf)]
        C["bv"] = U.tile([128, d_], BF16, name="bv", tag="bv", bufs=1)
        nc.scalar.dma_start(C["bv"][:], bvrep[:])
        C["ones"] = U.tile([128, 1], BF16, name="ones", tag="ones", bufs=1)
        nc.scalar.dma_start(C["ones"][:], onesb[:, 0:1])
        C["onesr"] = U.tile([1, 128], BF16, name="onesr", tag="onesr", bufs=1)
        nc.scalar.dma_start(C["onesr"][:], onesb[0:1, :])
        C["eps"] = U.tile([1, 1], F32, name="eps", tag="eps", bufs=1)
        nc.vector.memset(C["eps"][:], LN_EPS)
        # V resident tiles live across iterations: their ones column is
        # written once here and never touched by the per-iter V projection.
        C["vr"] = [U.tile([128, h_, he], BF16, name="vr", tag="vr", bufs=nst)
                   for _ in range(nst)]
        for st in range(nst):
            nc.vector.memset(C["vr"][st][:, :, hd:hd + 1], 1.0)
        # num ring is pre-zeroed so stale-block zeroing (0 * finite) is safe
        C["numz"] = [U.tile([128, npos * QCHUNK], BF16, name="numz",
                            tag="num", bufs=3) for _ in range(3)]
        for t in C["numz"]:
            nc.vector.memset(t[:], 0.0)
        return C

    def emit_body(tc, P, C):
        U = P["u"]

        # ---- weights: K+Q share a tag ring, V+Out share a tag ring ----
        wkt = U.tile([128, ndc, d_], BF16, name="wkt", tag="wkq", bufs=1)
        for wh in range(4):
            nc.sync.dma_start(
                wkt[:, :, wh * 256:(wh + 1) * 256],
                wqkv[:, d_ + wh * 256:d_ + (wh + 1) * 256]
                .rearrange("(c p) f -> p c f", p=128))
        wk = [wkt[:, dc, :] for dc in range(ndc)]
        wv = None                      # loaded after quarter 0's x tiles

        # ---- resident per-iteration tensors --------------------------
        kr = [U.tile([128, s_], BF16, name="kr", tag="kr", bufs=nf)
              for _ in range(nf)]
        qr = [U.tile([128, sq], BF16, name="qr", tag="qz", bufs=nf)
              for _ in range(nf)]
        ar = [U.tile([128, sq], BF16, name="ar", tag="ar", bufs=nf)
              for _ in range(nf)]
        xqt = U.tile([128, nf, sq], BF16, name="xqt", tag="xq", bufs=1)
        xq = [xqt[:, f, :] for f in range(nf)]
        mkt = U.tile([128, ntt, QCHUNK], BF16, name="mkt", tag="mk", bufs=1)
        C["mk"] = [mkt[:, i, :] for i in range(ntt)]
        vr = C["vr"]

        # ---- P1: stream x quarters; K^T + V fused; then Q^T ----------
        if do_p1:
            for quarter in range(nqr):
                cs = slice(quarter * 512, (quarter + 1) * 512)
                xtt = U.tile([128, ndc, 512], BF16, name="xtt", tag="xt",
                             bufs=2)
                if quarter == 0:
                    for dh in range(2):
                        nc.sync.dma_start(
                            xtt[:, dh * (ndc // 2):(dh + 1) * (ndc // 2), :],
                            xT[dh * 512:(dh + 1) * 512, cs]
                            .rearrange("(c p) s -> p c s", p=128))
                else:
                    nc.sync.dma_start(
                        xtt[:],
                        xT[:, cs].rearrange("(c p) s -> p c s", p=128))
                xt = [xtt[:, dc, :] for dc in range(ndc)]
                if quarter == 1:
                    # bulk non-urgent loads queue behind this quarter's x
                    # tiles so they cannot starve the startup-critical path
                    nc.sync.dma_start(
                        xqt[:], xqT.rearrange("(c p) s -> p c s", p=128))
                    nc.sync.dma_start(mkt[:],
                                      maskT.rearrange("t p q -> p t q"))
                if quarter == 0:
                    wvt = U.tile([128, ndc, d_], BF16, name="wvt", tag="wvo",
                                 bufs=1)
                    nc.sync.dma_start(
                        wvt[:],
                        wqkv[:, 2 * d_:3 * d_].rearrange("(c p) f -> p c f",
                                                         p=128))
                    wv = [wvt[:, dc, :] for dc in range(ndc)]
                # K^T: feature-major, ACT applies bias via Copy
                for f0 in range(0, nf, 2):
                    ps = P["ps"].tile([128, 1024], F32, name="psK", tag="sc")
                    for dc in range(ndc):
                        for f in (f0, f0 + 1):
                            nc.tensor.matmul(
                                ps[:, (f - f0) * 512:(f - f0) * 512 + 512],
                                wk[dc][:, f * 128:(f + 1) * 128],
                                xt[dc][:],
                                start=(dc == 0), stop=(dc == ndc - 1))
                    for f in (f0, f0 + 1):
                        nc.scalar.activation(
                            kr[f][:, cs],
                            ps[:, (f - f0) * 512:(f - f0) * 512 + 512],
                            ACTF.Identity, bias=C["bk"][f][:])
                # V: token-major with resident ones column
                for sl0 in range(0, 4, 2):
                    for fc in range(2):
                        ps = P["ps"].tile([128, 1024], F32, name="psV",
                                          tag="sc")
                        for dc in range(ndc):
                            for sl in (sl0, sl0 + 1):
                                nc.tensor.matmul(
                                    ps[:, (sl - sl0) * 512:(sl - sl0) * 512 + 512],
                                    xt[dc][:, sl * 128:(sl + 1) * 128],
                                    wv[dc][:, fc * 512:(fc + 1) * 512],
                                    start=(dc == 0), stop=(dc == ndc - 1))
                        for sl in (sl0, sl0 + 1):
                            st_i = quarter * 4 + sl
                            nc.vector.tensor_add(
                                vr[st_i][:, fc * (512 // hd):(fc + 1) * (512 // hd), 0:hd],
                                ps[:, (sl - sl0) * 512:(sl - sl0) * 512 + 512]
                                .rearrange("p (h e) -> p h e", e=hd),
                                C["bv"][:, fc * 512:(fc + 1) * 512]
                                .rearrange("p (h e) -> p h e", e=hd))
            # wq for the P2-interleaved Q projection (reuses wk's ring slot)
            wqt = U.tile([128, ndc, d_], BF16, name="wqt", tag="wkq", bufs=1)
            nc.sync.dma_start(
                wqt[:], wqkv[:, 0:d_].rearrange("(c p) f -> p c f", p=128))
            wq = [wqt[:, dc, :] for dc in range(ndc)]

        # ---- P2: attention, fully SBUF-resident, delay-1 pipeline ----
        if do_p2:
            def emit_av(att, h, first, prev):
                tp, nump = prev
                for j in pair_js:
                    if tp not in kt_sets[j]:
                        continue
                    jj = (j // 2) * 512
                    nc.tensor.matmul(
                        att[0:he, jj:jj + 512],
                        vr[tp][:, h, :],
                        nump[:, jj:jj + 512],
                        start=first[j], stop=(tp == kt_lists[j][-1]))
                    first[j] = False

            def emit_epilogue_a(hp, attp):
                # denominator chain, issued on DVE right at the head
                # boundary so its latency hides under the next head's QKs
                den = U.tile([1, sq], BF16, name="den", tag="den", bufs=1)
                nc.vector.tensor_scalar_add(den[:], attp[hd:hd + 1, :],
                                            SM_EPS)
                nc.vector.reciprocal(den[:], den[:])
                return den

            def emit_epilogue(hp, attp, den):
                # normalize head hp; overlaps the next head's QK stream
                fp, halfp = hp // 2, hp % 2
                rp = P["ps"].tile([128, 1024], F32, name="rp", tag="sc")
                for c in range(sq // 512):
                    nc.tensor.matmul(rp[0:hd, c * 512:(c + 1) * 512],
                                     C["onesr"][:, 0:hd],
                                     den[:, c * 512:(c + 1) * 512],
                                     start=True, stop=True)
                # DVE may read only one PSUM operand: stage rp through SBUF
                rps = U.tile([hd, sq], BF16, name="rps", tag="rps", bufs=1)
                nc.vector.tensor_scalar_add(rps[:], rp[0:hd, 0:sq], 0.0)
                if halfp == 0:
                    nc.vector.tensor_mul(ar[fp][0:hd, :], attp[0:hd, :],
                                         rps[:])
                else:
                    an = U.tile([hd, sq], BF16, name="an", tag="an", bufs=1)
                    nc.vector.tensor_mul(an[:], attp[0:hd, :], rps[:])
                    nc.sync.dma_start(ar[fp][hd:2 * hd, :], an[:])

            def emit_qproj(fq, ps, dc0, dc1):
                # partial Q^T projection for pair fq: contraction steps
                # [dc0, dc1) into the two 512-col bank groups of ps
                for dc in range(dc0, dc1):
                    for c in range(sq // 512):
                        nc.tensor.matmul(
                            ps[:, c * 512:(c + 1) * 512],
                            wq[dc][:, fq * 128:(fq + 1) * 128],
                            xq[dc][:, c * 512:(c + 1) * 512],
                            start=(dc == 0), stop=(dc == ndc - 1))

            if do_p1:
                ps = P["ps"].tile([128, 1024], F32, name="psQ", tag="sc")
                emit_qproj(0, ps, 0, ndc)
                nc.vector.tensor_scalar_add(qr[0][:], ps[:, 0:sq],
                                            C["bq"][0][:])
            pend = None                              # (h, att) awaiting epilogue
            for h in range(h_):
                f, half = h // 2, h % 2
                pb = half * hd                       # partition base
                # next pair's Q projection drips into this head's stream
                fq = f + 1 if (half == 1 and f + 1 < nf and do_p1) else None
                qps = None
                qdone = False
                att = P["att"].tile([128, npos * QCHUNK], F32, name="att",
                                    tag="att")
                if pend is not None:
                    pend = (pend[0], pend[1], emit_epilogue_a(*pend))
                first = {j: True for j in pair_js}
                prevs = []                           # pending (t, num_tile)
                ti = 0
                for t in all_kt:
                    jlist = [j for j in pair_js if t in kt_sets[j]]
                    if not jlist:
                        continue
                    sc = P["ps"].tile([128, 1024], F32, name="sc", tag="sc")
                    for j in jlist:
                        jj = (j // 2) * 512
                        nc.tensor.matmul(
                            sc[:, jj:jj + 512],
                            kr[f][pb:pb + hd, t * KTILE:(t + 1) * KTILE],
                            qr[f][pb:pb + hd, j * QCHUNK:(j + 2) * QCHUNK],
                            start=True, stop=True)
                    num = U.tile([128, npos * QCHUNK], BF16, name="num",
                                 tag="num", bufs=3)
                    p0, p1 = exp_runs[t]
                    nc.scalar.activation(
                        num[:, p0 * QCHUNK:p1 * QCHUNK],
                        sc[:, p0 * QCHUNK:p1 * QCHUNK],
                        ACTF.Exp, scale=inv_sqrt_hd)
                    for p in range(npos):
                        if (p, t) in tt_idx:
                            nc.vector.tensor_mul(
                                num[:, p * QCHUNK:(p + 1) * QCHUNK],
                                num[:, p * QCHUNK:(p + 1) * QCHUNK],
                                C["mk"][tt_idx[(p, t)]][:])
                    if len(prevs) == 2:
                        emit_av(att, h, first, prevs.pop(0))
                    prevs.append((t, num))
                    ti += 1
                    if ti == 4 and pend is not None:
                        emit_epilogue(*pend)
                        pend = None
                    if fq is not None and not qdone:
                        # two contiguous half-bursts keep the sc ring safe:
                        # every qps instruction precedes any later slot waiter
                        if ti == 5:
                            qps = P["ps"].tile([128, 1024], F32, name="psQ",
                                               tag="sc")
                            emit_qproj(fq, qps, 0, ndc // 2)
                        elif ti == 6 and qps is not None:
                            emit_qproj(fq, qps, ndc // 2, ndc)
                            nc.vector.tensor_scalar_add(
                                qr[fq][:], qps[:, 0:sq], C["bq"][fq][:])
                            qdone = True
                for pv in prevs:
                    emit_av(att, h, first, pv)
                if pend is not None:                 # very short tile lists
                    emit_epilogue(*pend)
                    pend = None
                if fq is not None and not qdone:     # short tile lists
                    if qps is None:
                        qps = P["ps"].tile([128, 1024], F32, name="psQ",
                                           tag="sc")
                        emit_qproj(fq, qps, 0, ndc)
                    else:
                        emit_qproj(fq, qps, ndc // 2, ndc)
                    nc.vector.tensor_scalar_add(qr[fq][:], qps[:, 0:sq],
                                                C["bq"][fq][:])
                pend = (h, att)
            if pend is not None:
                emit_epilogue(pend[0], pend[1],
                              emit_epilogue_a(pend[0], pend[1]))

        # ---- P3: out-projection + bias + residual + LN stats ----------
        if do_p34:
            wot = U.tile([128, ndc, d_], BF16, name="wot", tag="wvo", bufs=1)
            nc.sync.dma_start(
                wot[:], wout.rearrange("(c p) f -> p c f", p=128))
            wo = [wot[:, dc, :] for dc in range(ndc)]
            sum_ps = P["att"].tile([128, sq], F32, name="sum", tag="att")
            ssq_ps = P["att"].tile([128, sq], F32, name="ssq", tag="att")
            zr = [U.tile([128, sq], BF16, name="zr", tag="qz", bufs=nf)
                  for _ in range(nf)]
            for f in range(nf):
                for c in range(sq // 512):
                    ps = P["ps"].tile([128, 1024], F32, name="psO", tag="sc")
                    for dc in range(ndc):
                        nc.tensor.matmul(
                            ps[:, 0:512],
                            wo[dc][:, f * 128:(f + 1) * 128],
                            ar[dc][:, c * 512:(c + 1) * 512],
                            start=(dc == 0), stop=(dc == ndc - 1))
                    nc.vector.scalar_tensor_tensor(
                        zr[f][:, c * 512:(c + 1) * 512],
                        in0=ps[:, 0:512],
                        scalar=C["bo"][f][:],
                        in1=xq[f][:, c * 512:(c + 1) * 512],
                        op0=ALU.add, op1=ALU.add)
                sqz = U.tile([128, sq], BF16, name="sqz", tag="num", bufs=3)
                nc.scalar.activation(sqz[:], zr[f][:], ACTF.Square)
                for c in range(sq // 512):
                    nc.tensor.matmul(sum_ps[0:1, c * 512:(c + 1) * 512],
                                     C["ones"][:],
                                     zr[f][:, c * 512:(c + 1) * 512],
                                     start=(f == 0), stop=(f == nf - 1))
                    nc.tensor.matmul(ssq_ps[0:1, c * 512:(c + 1) * 512],
                                     C["ones"][:],
                                     sqz[:, c * 512:(c + 1) * 512],
                                     start=(f == 0), stop=(f == nf - 1))

        # ---- P4: LayerNorm normalize ---------------------------------
        if do_p34:
            lmean = U.tile([1, sq], BF16, name="lmean", tag="lmean", bufs=1)
            lrstd = U.tile([1, sq], BF16, name="lrstd", tag="lrstd", bufs=1)
            msq = U.tile([1, sq], BF16, name="msq", tag="lnsc", bufs=1)
            m2 = U.tile([1, sq], BF16, name="m2", tag="lnsc2", bufs=1)
            nc.vector.tensor_scalar_mul(lmean[:], sum_ps[0:1, :], 1.0 / d_)
            nc.vector.tensor_scalar_mul(msq[:], ssq_ps[0:1, :], 1.0 / d_)
            nc.vector.tensor_mul(m2[:], lmean[:], lmean[:])
            nc.vector.tensor_sub(msq[:], msq[:], m2[:])        # var
            nc.scalar.activation(m2[:], msq[:], ACTF.Sqrt,
                                 bias=C["eps"][:])             # std
            nc.vector.reciprocal(lrstd[:], m2[:])
            mrep_ps = P["ps"].tile([128, 1024], F32, name="mrep", tag="sc")
            rrep_ps = P["ps"].tile([128, 1024], F32, name="rrep", tag="sc")
            for c in range(sq // 512):
                nc.tensor.matmul(mrep_ps[:, c * 512:(c + 1) * 512],
                                 C["onesr"][:, 0:128],
                                 lmean[:, c * 512:(c + 1) * 512],
                                 start=True, stop=True)
                nc.tensor.matmul(rrep_ps[:, c * 512:(c + 1) * 512],
                                 C["onesr"][:, 0:128],
                                 lrstd[:, c * 512:(c + 1) * 512],
                                 start=True, stop=True)
            mrep = U.tile([128, sq], BF16, name="mrepS", tag="rep", bufs=2)
            rrep = U.tile([128, sq], BF16, name="rrepS", tag="rep", bufs=2)
            nc.scalar.activation(mrep[:], mrep_ps[:, 0:sq], ACTF.Copy)
            nc.scalar.activation(rrep[:], rrep_ps[:, 0:sq], ACTF.Copy)
            for f in range(nf):
                t1 = U.tile([128, sq], BF16, name="t1", tag="t1", bufs=3)
                nc.vector.tensor_sub(t1[:], zr[f][:], mrep[:])
                nc.vector.tensor_mul(t1[:], t1[:], rrep[:])
                t2 = U.tile([128, sq], BF16, name="t2", tag="num", bufs=3)
                nc.scalar.activation(t2[:], t1[:], ACTF.Identity,
                                     bias=C["be"][f][:], scale=C["ga"][f][:])
                nc.sync.dma_start(yT[f * 128:(f + 1) * 128, :], t2[:])

    from contextlib import ExitStack
    with tile.TileContext(nc) as tc:
        with ExitStack() as ctx:
            P = {
                "u": ctx.enter_context(tc.tile_pool(name="u", bufs=2)),
                "ps": ctx.enter_context(
                    tc.tile_pool(name="ps", bufs=2, space="PSUM")),
                "att": ctx.enter_context(
                    tc.tile_pool(name="att", bufs=2, space="PSUM")),
            }
            ctx_lp = nc.allow_low_precision(reason="bf16 matmul operand chain")
            ctx_lp.__enter__()
            C = emit_consts(tc, P)
            if n_iters > 1:
                with tc.For_i(0, n_iters, 1):
                    emit_body(tc, P, C)
            else:
                emit_body(tc, P, C)
            ctx_lp.__exit__(None, None, None)
    nc.compile()
    return nc


# ----------------------------------------------------------------------------
# Host wrapper
# ----------------------------------------------------------------------------

_CACHE = {}


def _get_program(pattern_key, kt_lists, tt_blocks, exp_runs=None, n_iters=1,
                 timing_mode=False, phases=("p1", "p2", "p34")):
    key = (pattern_key, n_iters, timing_mode, tuple(phases))
    if key not in _CACHE:
        if exp_runs is None:
            npos = len(kt_lists)
            exp_runs = {}
            for p, kl in enumerate(kt_lists):
                for t in kl:
                    lo, hi = exp_runs.get(t, (npos, 0))
                    exp_runs[t] = (min(lo, p), max(hi, p + 1))
        _CACHE[key] = _build_program((B, S, D, H), kt_lists, tt_blocks,
                                     exp_runs, n_iters=n_iters,
                                     timing_mode=timing_mode, phases=phases)
    return _CACHE[key]


def _prep_inputs(x, attn_mask, W_qkv, b_qkv, W_out, b_out, gamma, beta,
                 chunk_at, tt_blocks):
    b_, s_, d_ = x.shape
    f32 = np.float32
    bf16 = ml_dtypes.bfloat16
    in_maps = []
    qsels = []
    bvrep = np.ascontiguousarray(
        np.broadcast_to(b_qkv[2 * d_:3 * d_][None, :], (128, d_))
    ).astype(bf16)
    wqkv_c = np.ascontiguousarray(W_qkv).astype(bf16)
    wout_c = np.ascontiguousarray(W_out).astype(bf16)
    nf = d_ // 128
    b_qkv = np.asarray(b_qkv, f32)
    vecs = ([b_qkv[f * 128:(f + 1) * 128] for f in range(nf)] +
            [b_qkv[d_ + f * 128:d_ + (f + 1) * 128] for f in range(nf)] +
            [np.asarray(b_out, f32)[f * 128:(f + 1) * 128] for f in range(nf)] +
            [np.asarray(gamma, f32)[f * 128:(f + 1) * 128] for f in range(nf)] +
            [np.asarray(beta, f32)[f * 128:(f + 1) * 128] for f in range(nf)])
    bpack = np.ascontiguousarray(np.stack(vecs, axis=1), dtype=f32)
    onesb = np.ones((128, 128), bf16)
    hd = d_ // H
    for core in range(N_CORES):
        bb, sh = core // 2, core % 2
        chunks = chunk_at[bb, sh]
        qsel = np.concatenate(
            [np.arange(c * QCHUNK, (c + 1) * QCHUNK) for c in chunks])
        qsels.append(qsel)
        xT_ = np.ascontiguousarray(x[bb].T).astype(bf16)
        xqT_ = np.ascontiguousarray(x[bb][qsel].T).astype(bf16)
        if tt_blocks:
            mt = np.stack([
                np.exp(np.ascontiguousarray(
                    attn_mask[bb,
                              chunks[p] * QCHUNK:(chunks[p] + 1) * QCHUNK,
                              t * KTILE:(t + 1) * KTILE].T,
                    dtype=np.float64) / np.sqrt(hd))
                for (p, t) in tt_blocks]).astype(bf16)
        else:
            mt = np.zeros((1, KTILE, QCHUNK), bf16)
        in_maps.append({
            "xT": xT_, "xqT": xqT_,
            "wqkv": wqkv_c, "wout": wout_c,
            "bpack": bpack,
            "bvrep": bvrep, "maskT": mt,
            "onesb": onesb,
        })
    return in_maps, qsels


def kernel(x, attn_mask, W_qkv, b_qkv, W_out, b_out, gamma, beta,
           n_iters=1):
    x = np.asarray(x, np.float32)
    attn_mask = np.asarray(attn_mask, np.float32)
    chunk_at, kt_lists, tt_blocks, exp_runs = _mask_pattern(attn_mask, B, S)
    pattern_key = (tuple(tuple(k) for k in kt_lists), tuple(tt_blocks),
                   tuple(sorted(exp_runs.items())))
    nc = _get_program(pattern_key, kt_lists, tt_blocks, exp_runs,
                      n_iters=n_iters)
    in_maps, qsels = _prep_inputs(
        x, attn_mask, np.asarray(W_qkv), np.asarray(b_qkv),
        np.asarray(W_out), np.asarray(b_out), np.asarray(gamma),
        np.asarray(beta), chunk_at, tt_blocks)
    res = run_bass_kernel_spmd(nc, in_maps, list(range(N_CORES)))
    out = np.empty((B, S, D), np.float32)
    for core in range(N_CORES):
        bb = core // 2
        out[bb, qsels[core]] = np.asarray(
            res.results[core]["yT"], dtype=np.float32).T
    return out


# revision 25
# speedup vs baseline: 1.1345x; 1.1345x over previous
"""Trainium2 Bass kernel for a causal self-attention block.

Reference computation (fp32):
    qkv = x @ W_qkv + b_qkv ; q,k,v = split(qkv)
    scores = (q @ k.T + mask) / sqrt(hd)
    wts = exp(scores) / (sum(exp(scores)) + 1e-9)
    y = (wts @ v) @ W_out + b_out
    out = LayerNorm(x + y) * gamma + beta

Sharding: 8 cores = 4 batches x 2 query-shards; the query chunks are
interleaved between the two shards of a batch so causal block-skipping
is balanced.  The block pattern is derived from the actual mask on the
host, so non-causal masks degrade gracefully to the dense kernel.

v2 design (vs the spill-based v1): everything lives in SBUF in bf16 —
K^T / Q^T / V / attn^T / z are resident, there are no HBM spills at all.
P1 streams x^T tiles once and produces K and V from the same tiles
(fused loops), then Q from the resident x_q.  P2 is fully SBUF-resident
per head with software pipelining: attn@V runs one k-tile behind the
exp so the PE never waits on the activation engine.  The softmax
denominator rides along as a ones column in V; exp covers only the
exact needed position prefix per k-tile (stale columns are zeroed by
the mask multiply; the num ring is pre-zeroed once so stale data is
always finite).  Masks, biases and the V ones-column are loop invariant
and hoisted out of the For_i timing loop; weights are double-buffered
across iterations via shared tags (K+Q, V+Out).  DMA issue is split
between the SP and Activation hardware queues.
"""

import numpy as np
import ml_dtypes

import concourse.bass as bass
import concourse.tile as tile
from concourse import bacc, mybir
from concourse.bass_utils import run_bass_kernel_spmd

# Problem dims (hardcoded per harness contract)
B, S, D, H = 4, 2048, 1024, 16
N_CORES = 8
QCHUNK = 256                     # q columns per position slot
KTILE = 128                      # k rows per tile
NEG_THRESH = -1.0e8              # mask <= this  =>  exp((qk+m)/8) == 0 in fp32
SM_EPS = 1.0e-9
LN_EPS = 1.0e-5

F32 = mybir.dt.float32
BF16 = mybir.dt.bfloat16

ALU = mybir.AluOpType
ACTF = mybir.ActivationFunctionType


# ----------------------------------------------------------------------------
# Host-side mask analysis / shard assignment
# ----------------------------------------------------------------------------

def _mask_pattern(attn_mask, b_, s_):
    """Classify mask blocks and build a core-uniform block schedule.

    Returns (chunk_at[b, shard, pos], kt_lists[pos] -> list of ktiles,
    tt_blocks -> ordered (pos, ktile) needing a mask multiply,
    exp_runs[t] -> (p0, p1) position-slot range the exp must cover).
    """
    nq = s_ // QCHUNK
    nk = s_ // KTILE
    m = attn_mask.reshape(b_, nq, QCHUNK, nk, KTILE)
    bmax = m.max(axis=(2, 4))
    bmin = m.min(axis=(2, 4))
    skip = bmax <= NEG_THRESH                    # contributes exactly 0
    zero = (bmin == 0.0) & (bmax == 0.0)         # no mask add needed
    needed = ~skip                               # [b, nq, nk]
    counts = needed.sum(axis=2)                  # [b, nq]

    order = np.argsort(-counts, axis=1, kind="stable")   # chunks by desc count
    npos = nq // 2
    chunk_at = np.zeros((b_, 2, npos), np.int64)
    chunk_at[:, 0, :] = order[:, 0::2]
    chunk_at[:, 1, :] = order[:, 1::2]

    kt_lists = []
    tt_blocks = []
    un_list = []
    tt_list = []
    for p in range(npos):
        un = np.zeros(nk, bool)
        for bb in range(b_):
            for sh in range(2):
                un |= needed[bb, chunk_at[bb, sh, p]]
        kt_lists.append(np.nonzero(un)[0].tolist())
        un_list.append(un)
        tt_list.append(np.zeros(nk, bool))
    # PSUM accumulation groups are per 512-col bank: fuse position pairs into
    # one group by unioning their k-tile lists; the extra blocks are masked.
    for j in range(0, npos - 1, 2):
        un2 = un_list[j] | un_list[j + 1]
        kt_lists[j] = kt_lists[j + 1] = np.nonzero(un2)[0].tolist()
        for p in (j, j + 1):
            # mask multiply needed wherever a core computes the block but its
            # mask is not identically zero there (incl. dead padding)
            any_tt = np.zeros(nk, bool)
            for bb in range(b_):
                for sh in range(2):
                    c = chunk_at[bb, sh, p]
                    any_tt |= un2 & ~(needed[bb, c] & zero[bb, c])
            tt_list[p] = any_tt
    for p in range(npos):
        for t in kt_lists[p]:
            if tt_list[p][t]:
                tt_blocks.append((p, t))

    # exp coverage per k-tile: the position-slot run [p0, p1) the exp reads.
    # Exact (only slots that truly need t) when those slots are contiguous;
    # otherwise the full pair-computed range.  Consumed-but-uncovered blocks
    # hold stale-but-finite data and are zeroed by the mask multiply (they
    # are in tt by construction).
    exp_runs = {}
    all_t = sorted({t for kl in kt_lists for t in kl})
    for t in all_t:
        ps = [p for p in range(npos) if un_list[p][t]]
        if ps and ps == list(range(ps[0], ps[-1] + 1)):
            exp_runs[t] = (ps[0], ps[-1] + 1)
        else:
            pr = [p for p in range(npos) if t in kt_lists[p]]
            exp_runs[t] = (min(pr), max(pr) + 1)
    return chunk_at, kt_lists, tt_blocks, exp_runs


# ----------------------------------------------------------------------------
# Device program
# ----------------------------------------------------------------------------

def _build_program(dims, kt_lists, tt_blocks, exp_runs,
                   n_iters=1, timing_mode=False,
                   phases=("p1", "p2", "p34")):
    """Emit the per-core Bass/Tile program (identical for all cores)."""
    b_, s_, d_, h_ = dims
    hd = d_ // h_                 # 64
    he = hd + 1                   # head dim + ones column
    sq = s_ // 2                  # queries per core
    npos = sq // QCHUNK           # 4
    nf = d_ // 128                # feature tiles
    nst = s_ // 128               # s tiles (token-major V) == n ktiles
    ndc = d_ // 128               # contraction d tiles
    nqr = s_ // 512               # x streaming quarters
    ntt = max(1, len(tt_blocks))
    tt_idx = {pt: i for i, pt in enumerate(tt_blocks)}
    kt_sets = [set(k) for k in kt_lists]
    pair_js = [j for j in range(0, npos, 2)]
    all_kt = sorted(set().union(*kt_sets)) if kt_lists else []
    inv_sqrt_hd = 1.0 / float(np.sqrt(hd))

    nc = bacc.Bacc("TRN2", target_bir_lowering=False, debug=False,
                   num_devices=N_CORES)

    # I/O.  In timing_mode the big inputs are internal DRAM scratch
    # (uninitialised) so per-call host->device transfer stays tiny.
    big = "Internal" if timing_mode else "ExternalInput"
    xT = nc.dram_tensor("xT", [d_, s_], BF16, kind=big).ap()
    xqT = nc.dram_tensor("xqT", [d_, sq], BF16, kind=big).ap()
    wqkv = nc.dram_tensor("wqkv", [d_, 3 * d_], BF16, kind=big).ap()
    wout = nc.dram_tensor("wout", [d_, d_], BF16, kind=big).ap()
    bpack = nc.dram_tensor("bpack", [128, 5 * nf], F32,
                           kind="ExternalInput").ap()
    bvrep = nc.dram_tensor("bvrep", [128, d_], BF16, kind="ExternalInput").ap()
    maskT = nc.dram_tensor("maskT", [ntt, KTILE, QCHUNK], BF16,
                           kind="ExternalInput").ap()
    onesb = nc.dram_tensor("onesb", [128, 128], BF16,
                           kind="ExternalInput").ap()
    yT = nc.dram_tensor("yT", [d_, sq], BF16, kind="ExternalOutput").ap()

    do_p1 = "p1" in phases
    do_p2 = "p2" in phases
    do_p34 = "p34" in phases

    def emit_consts(tc, P):
        """Loop-invariant data: biases, masks, ones, V ones-column."""
        U = P["u"]
        C = {}
        bpt = U.tile([128, 5 * nf], F32, name="bpt", tag="bpt", bufs=1)
        nc.scalar.dma_start(bpt[:], bpack[:])
        for i, nm in enumerate(("bq", "bk", "bo", "ga", "be")):
            C[nm] = [bpt[:, i * nf + f:i * nf + f + 1] for f in range(nf)]
        C["bv"] = U.tile([128, d_], BF16, name="bv", tag="bv", bufs=1)
        nc.scalar.dma_start(C["bv"][:], bvrep[:])
        C["ones"] = U.tile([128, 1], BF16, name="ones", tag="ones", bufs=1)
        nc.scalar.dma_start(C["ones"][:], onesb[:, 0:1])
        C["onesr"] = U.tile([1, 128], BF16, name="onesr", tag="onesr", bufs=1)
        nc.scalar.dma_start(C["onesr"][:], onesb[0:1, :])
        C["eps"] = U.tile([1, 1], F32, name="eps", tag="eps", bufs=1)
        nc.vector.memset(C["eps"][:], LN_EPS)
        # V resident tiles live across iterations: their ones column is
        # written once here and never touched by the per-iter V projection.
        C["vr"] = [U.tile([128, h_, he], BF16, name="vr", tag="vr", bufs=nst)
                   for _ in range(nst)]
        for st in range(nst):
            nc.vector.memset(C["vr"][st][:, :, hd:hd + 1], 1.0)
        # num ring is pre-zeroed so stale-block zeroing (0 * finite) is safe
        C["numz"] = [U.tile([128, npos * QCHUNK], BF16, name="numz",
                            tag="num", bufs=3) for _ in range(3)]
        for t in C["numz"]:
            nc.vector.memset(t[:], 0.0)
        return C

    def emit_body(tc, P, C):
        U = P["u"]

        # ---- weights: K+Q share a tag ring, V+Out share a tag ring ----
        wkt = U.tile([128, ndc, d_], BF16, name="wkt", tag="wkq", bufs=1)
        for wh in range(4):
            nc.sync.dma_start(
                wkt[:, :, wh * 256:(wh + 1) * 256],
                wqkv[:, d_ + wh * 256:d_ + (wh + 1) * 256]
                .rearrange("(c p) f -> p c f", p=128))
        wk = [wkt[:, dc, :] for dc in range(ndc)]
        wv = None                      # loaded after quarter 0's x tiles

        # ---- resident per-iteration tensors --------------------------
        kr = [U.tile([128, s_], BF16, name="kr", tag="kr", bufs=nf)
              for _ in range(nf)]
        qr = [U.tile([128, sq], BF16, name="qr", tag="qz", bufs=nf)
              for _ in range(nf)]
        ar = [U.tile([128, sq], BF16, name="ar", tag="ar", bufs=nf)
              for _ in range(nf)]
        xqt = U.tile([128, nf, sq], BF16, name="xqt", tag="xq", bufs=1)
        xq = [xqt[:, f, :] for f in range(nf)]
        mkt = U.tile([128, ntt, QCHUNK], BF16, name="mkt", tag="mk", bufs=1)
        C["mk"] = [mkt[:, i, :] for i in range(ntt)]
        vr = C["vr"]

        # ---- P1: stream x quarters; K^T + V fused; then Q^T ----------
        if do_p1:
            for quarter in range(nqr):
                cs = slice(quarter * 512, (quarter + 1) * 512)
                xtt = U.tile([128, ndc, 512], BF16, name="xtt", tag="xt",
                             bufs=2)
                if quarter == 0:
                    for dh in range(2):
                        nc.sync.dma_start(
                            xtt[:, dh * (ndc // 2):(dh + 1) * (ndc // 2), :],
                            xT[dh * 512:(dh + 1) * 512, cs]
                            .rearrange("(c p) s -> p c s", p=128))
                else:
                    nc.sync.dma_start(
                        xtt[:],
                        xT[:, cs].rearrange("(c p) s -> p c s", p=128))
                xt = [xtt[:, dc, :] for dc in range(ndc)]
                if quarter == 1:
                    # bulk non-urgent loads queue behind this quarter's x
                    # tiles so they cannot starve the startup-critical path
                    nc.sync.dma_start(
                        xqt[:], xqT.rearrange("(c p) s -> p c s", p=128))
                    nc.sync.dma_start(mkt[:],
                                      maskT.rearrange("t p q -> p t q"))
                if quarter == 0:
                    wvt = U.tile([128, ndc, d_], BF16, name="wvt", tag="wvo",
                                 bufs=1)
                    nc.sync.dma_start(
                        wvt[:],
                        wqkv[:, 2 * d_:3 * d_].rearrange("(c p) f -> p c f",
                                                         p=128))
                    wv = [wvt[:, dc, :] for dc in range(ndc)]
                # K^T: feature-major, ACT applies bias via Copy
                for f0 in range(0, nf, 2):
                    ps = P["ps"].tile([128, 1024], F32, name="psK", tag="sc")
                    for dc in range(ndc):
                        for f in (f0, f0 + 1):
                            nc.tensor.matmul(
                                ps[:, (f - f0) * 512:(f - f0) * 512 + 512],
                                wk[dc][:, f * 128:(f + 1) * 128],
                                xt[dc][:],
                                start=(dc == 0), stop=(dc == ndc - 1))
                    for f in (f0, f0 + 1):
                        nc.scalar.activation(
                            kr[f][:, cs],
                            ps[:, (f - f0) * 512:(f - f0) * 512 + 512],
                            ACTF.Identity, bias=C["bk"][f][:])
                # V: token-major with resident ones column
                for sl0 in range(0, 4, 2):
                    for fc in range(2):
                        ps = P["ps"].tile([128, 1024], F32, name="psV",
                                          tag="sc")
                        for dc in range(ndc):
                            for sl in (sl0, sl0 + 1):
                                nc.tensor.matmul(
                                    ps[:, (sl - sl0) * 512:(sl - sl0) * 512 + 512],
                                    xt[dc][:, sl * 128:(sl + 1) * 128],
                                    wv[dc][:, fc * 512:(fc + 1) * 512],
                                    start=(dc == 0), stop=(dc == ndc - 1))
                        for sl in (sl0, sl0 + 1):
                            st_i = quarter * 4 + sl
                            nc.vector.tensor_add(
                                vr[st_i][:, fc * (512 // hd):(fc + 1) * (512 // hd), 0:hd],
                                ps[:, (sl - sl0) * 512:(sl - sl0) * 512 + 512]
                                .rearrange("p (h e) -> p h e", e=hd),
                                C["bv"][:, fc * 512:(fc + 1) * 512]
                                .rearrange("p (h e) -> p h e", e=hd))
            # wq for the P2-interleaved Q projection (reuses wk's ring slot)
            wqt = U.tile([128, ndc, d_], BF16, name="wqt", tag="wkq", bufs=1)
            nc.sync.dma_start(
                wqt[:], wqkv[:, 0:d_].rearrange("(c p) f -> p c f", p=128))
            wq = [wqt[:, dc, :] for dc in range(ndc)]

        # ---- P2: attention, fully SBUF-resident, delay-1 pipeline ----
        if do_p2:
            def emit_av(att, h, first, prev):
                tp, nump = prev
                for j in pair_js:
                    if tp not in kt_sets[j]:
                        continue
                    jj = (j // 2) * 512
                    nc.tensor.matmul(
                        att[0:he, jj:jj + 512],
                        vr[tp][:, h, :],
                        nump[:, jj:jj + 512],
                        start=first[j], stop=(tp == kt_lists[j][-1]))
                    first[j] = False

            def emit_epilogue(hp, attp):
                # normalize head hp by (denom + eps); runs overlapped with
                # the next head's QK stream
                fp, halfp = hp // 2, hp % 2
                den = U.tile([1, sq], BF16, name="den", tag="den", bufs=1)
                nc.vector.tensor_scalar_add(den[:], attp[hd:hd + 1, :],
                                            SM_EPS)
                nc.vector.reciprocal(den[:], den[:])
                rp = P["ps"].tile([128, 1024], F32, name="rp", tag="sc")
                for c in range(sq // 512):
                    nc.tensor.matmul(rp[0:hd, c * 512:(c + 1) * 512],
                                     C["onesr"][:, 0:hd],
                                     den[:, c * 512:(c + 1) * 512],
                                     start=True, stop=True)
                # DVE may read only one PSUM operand: stage rp through SBUF
                rps = U.tile([hd, sq], BF16, name="rps", tag="rps", bufs=1)
                nc.vector.tensor_scalar_add(rps[:], rp[0:hd, 0:sq], 0.0)
                if halfp == 0:
                    nc.vector.tensor_mul(ar[fp][0:hd, :], attp[0:hd, :],
                                         rps[:])
                else:
                    an = U.tile([hd, sq], BF16, name="an", tag="an", bufs=1)
                    nc.vector.tensor_mul(an[:], attp[0:hd, :], rps[:])
                    nc.sync.dma_start(ar[fp][hd:2 * hd, :], an[:])

            def emit_qproj(fq, ps, dc0, dc1):
                # partial Q^T projection for pair fq: contraction steps
                # [dc0, dc1) into the two 512-col bank groups of ps
                for dc in range(dc0, dc1):
                    for c in range(sq // 512):
                        nc.tensor.matmul(
                            ps[:, c * 512:(c + 1) * 512],
                            wq[dc][:, fq * 128:(fq + 1) * 128],
                            xq[dc][:, c * 512:(c + 1) * 512],
                            start=(dc == 0), stop=(dc == ndc - 1))

            if do_p1:
                ps = P["ps"].tile([128, 1024], F32, name="psQ", tag="sc")
                emit_qproj(0, ps, 0, ndc)
                nc.vector.tensor_scalar_add(qr[0][:], ps[:, 0:sq],
                                            C["bq"][0][:])
            pend = None                              # (h, att) awaiting epilogue
            for h in range(h_):
                f, half = h // 2, h % 2
                pb = half * hd                       # partition base
                # next pair's Q projection drips into this head's stream
                fq = f + 1 if (half == 1 and f + 1 < nf and do_p1) else None
                qps = None
                qdone = False
                att = P["att"].tile([128, npos * QCHUNK], F32, name="att",
                                    tag="att")
                first = {j: True for j in pair_js}
                prevs = []                           # pending (t, num_tile)
                ti = 0
                for t in all_kt:
                    jlist = [j for j in pair_js if t in kt_sets[j]]
                    if not jlist:
                        continue
                    sc = P["ps"].tile([128, 1024], F32, name="sc", tag="sc")
                    for j in jlist:
                        jj = (j // 2) * 512
                        nc.tensor.matmul(
                            sc[:, jj:jj + 512],
                            kr[f][pb:pb + hd, t * KTILE:(t + 1) * KTILE],
                            qr[f][pb:pb + hd, j * QCHUNK:(j + 2) * QCHUNK],
                            start=True, stop=True)
                    num = U.tile([128, npos * QCHUNK], BF16, name="num",
                                 tag="num", bufs=3)
                    p0, p1 = exp_runs[t]
                    nc.scalar.activation(
                        num[:, p0 * QCHUNK:p1 * QCHUNK],
                        sc[:, p0 * QCHUNK:p1 * QCHUNK],
                        ACTF.Exp, scale=inv_sqrt_hd)
                    for p in range(npos):
                        if (p, t) in tt_idx:
                            nc.vector.tensor_mul(
                                num[:, p * QCHUNK:(p + 1) * QCHUNK],
                                num[:, p * QCHUNK:(p + 1) * QCHUNK],
                                C["mk"][tt_idx[(p, t)]][:])
                    if len(prevs) == 2:
                        emit_av(att, h, first, prevs.pop(0))
                    prevs.append((t, num))
                    ti += 1
                    if ti == 2 and pend is not None:
                        emit_epilogue(*pend)
                        pend = None
                    if fq is not None and not qdone:
                        # two contiguous half-bursts keep the sc ring safe:
                        # every qps instruction precedes any later slot waiter
                        if ti == 3:
                            qps = P["ps"].tile([128, 1024], F32, name="psQ",
                                               tag="sc")
                            emit_qproj(fq, qps, 0, ndc // 2)
                        elif ti == 4 and qps is not None:
                            emit_qproj(fq, qps, ndc // 2, ndc)
                            nc.vector.tensor_scalar_add(
                                qr[fq][:], qps[:, 0:sq], C["bq"][fq][:])
                            qdone = True
                for pv in prevs:
                    emit_av(att, h, first, pv)
                if pend is not None:                 # very short tile lists
                    emit_epilogue(*pend)
                    pend = None
                if fq is not None and not qdone:     # short tile lists
                    if qps is None:
                        qps = P["ps"].tile([128, 1024], F32, name="psQ",
                                           tag="sc")
                        emit_qproj(fq, qps, 0, ndc)
                    else:
                        emit_qproj(fq, qps, ndc // 2, ndc)
                    nc.vector.tensor_scalar_add(qr[fq][:], qps[:, 0:sq],
                                                C["bq"][fq][:])
                pend = (h, att)
            if pend is not None:
                emit_epilogue(*pend)

        # ---- P3: out-projection + bias + residual + LN stats ----------
        if do_p34:
            wot = U.tile([128, ndc, d_], BF16, name="wot", tag="wvo", bufs=1)
            nc.sync.dma_start(
                wot[:], wout.rearrange("(c p) f -> p c f", p=128))
            wo = [wot[:, dc, :] for dc in range(ndc)]
            sum_ps = P["att"].tile([128, sq], F32, name="sum", tag="att")
            ssq_ps = P["att"].tile([128, sq], F32, name="ssq", tag="att")
            zr = [U.tile([128, sq], BF16, name="zr", tag="qz", bufs=nf)
                  for _ in range(nf)]
            for f in range(nf):
                for c in range(sq // 512):
                    ps = P["ps"].tile([128, 1024], F32, name="psO", tag="sc")
                    for dc in range(ndc):
                        nc.tensor.matmul(
                            ps[:, 0:512],
                            wo[dc][:, f * 128:(f + 1) * 128],
                            ar[dc][:, c * 512:(c + 1) * 512],
                            start=(dc == 0), stop=(dc == ndc - 1))
                    nc.vector.scalar_tensor_tensor(
                        zr[f][:, c * 512:(c + 1) * 512],
                        in0=ps[:, 0:512],
                        scalar=C["bo"][f][:],
                        in1=xq[f][:, c * 512:(c + 1) * 512],
                        op0=ALU.add, op1=ALU.add)
                sqz = U.tile([128, sq], BF16, name="sqz", tag="num", bufs=3)
                nc.scalar.activation(sqz[:], zr[f][:], ACTF.Square)
                for c in range(sq // 512):
                    nc.tensor.matmul(sum_ps[0:1, c * 512:(c + 1) * 512],
                                     C["ones"][:],
                                     zr[f][:, c * 512:(c + 1) * 512],
                                     start=(f == 0), stop=(f == nf - 1))
                    nc.tensor.matmul(ssq_ps[0:1, c * 512:(c + 1) * 512],
                                     C["ones"][:],
                                     sqz[:, c * 512:(c + 1) * 512],
                                     start=(f == 0), stop=(f == nf - 1))

        # ---- P4: LayerNorm normalize ---------------------------------
        if do_p34:
            lmean = U.tile([1, sq], BF16, name="lmean", tag="lmean", bufs=1)
            lrstd = U.tile([1, sq], BF16, name="lrstd", tag="lrstd", bufs=1)
            msq = U.tile([1, sq], BF16, name="msq", tag="lnsc", bufs=1)
            m2 = U.tile([1, sq], BF16, name="m2", tag="lnsc2", bufs=1)
            nc.vector.tensor_scalar_mul(lmean[:], sum_ps[0:1, :], 1.0 / d_)
            nc.vector.tensor_scalar_mul(msq[:], ssq_ps[0:1, :], 1.0 / d_)
            nc.vector.tensor_mul(m2[:], lmean[:], lmean[:])
            nc.vector.tensor_sub(msq[:], msq[:], m2[:])        # var
            nc.scalar.activation(m2[:], msq[:], ACTF.Sqrt,
                                 bias=C["eps"][:])             # std
            nc.vector.reciprocal(lrstd[:], m2[:])
            mrep_ps = P["ps"].tile([128, 1024], F32, name="mrep", tag="sc")
            rrep_ps = P["ps"].tile([128, 1024], F32, name="rrep", tag="sc")
            for c in range(sq // 512):
                nc.tensor.matmul(mrep_ps[:, c * 512:(c + 1) * 512],
                                 C["onesr"][:, 0:128],
                                 lmean[:, c * 512:(c + 1) * 512],
                                 start=True, stop=True)
                nc.tensor.matmul(rrep_ps[:, c * 512:(c + 1) * 512],
                                 C["onesr"][:, 0:128],
                                 lrstd[:, c * 512:(c + 1) * 512],
                                 start=True, stop=True)
            mrep = U.tile([128, sq], BF16, name="mrepS", tag="rep", bufs=2)
            rrep = U.tile([128, sq], BF16, name="rrepS", tag="rep", bufs=2)
            nc.scalar.activation(mrep[:], mrep_ps[:, 0:sq], ACTF.Copy)
            nc.scalar.activation(rrep[:], rrep_ps[:, 0:sq], ACTF.Copy)
            for f in range(nf):
                t1 = U.tile([128, sq], BF16, name="t1", tag="t1", bufs=3)
                nc.vector.tensor_sub(t1[:], zr[f][:], mrep[:])
                nc.vector.tensor_mul(t1[:], t1[:], rrep[:])
                t2 = U.tile([128, sq], BF16, name="t2", tag="num", bufs=3)
                nc.scalar.activation(t2[:], t1[:], ACTF.Identity,
                                     bias=C["be"][f][:], scale=C["ga"][f][:])
                nc.sync.dma_start(yT[f * 128:(f + 1) * 128, :], t2[:])

    from contextlib import ExitStack
    with tile.TileContext(nc) as tc:
        with ExitStack() as ctx:
            P = {
                "u": ctx.enter_context(tc.tile_pool(name="u", bufs=2)),
                "ps": ctx.enter_context(
                    tc.tile_pool(name="ps", bufs=2, space="PSUM")),
                "att": ctx.enter_context(
                    tc.tile_pool(name="att", bufs=2, space="PSUM")),
            }
            ctx_lp = nc.allow_low_precision(reason="bf16 matmul operand chain")
            ctx_lp.__enter__()
            C = emit_consts(tc, P)
            if n_iters > 1:
                with tc.For_i(0, n_iters, 1):
                    emit_body(tc, P, C)
            else:
                emit_body(tc, P, C)
            ctx_lp.__exit__(None, None, None)
    nc.compile()
    return nc


# ----------------------------------------------------------------------------
# Host wrapper
# ----------------------------------------------------------------------------

_CACHE = {}


def _get_program(pattern_key, kt_lists, tt_blocks, exp_runs=None, n_iters=1,
                 timing_mode=False, phases=("p1", "p2", "p34")):
    key = (pattern_key, n_iters, timing_mode, tuple(phases))
    if key not in _CACHE:
        if exp_runs is None:
            npos = len(kt_lists)
            exp_runs = {}
            for p, kl in enumerate(kt_lists):
                for t in kl:
                    lo, hi = exp_runs.get(t, (npos, 0))
                    exp_runs[t] = (min(lo, p), max(hi, p + 1))
        _CACHE[key] = _build_program((B, S, D, H), kt_lists, tt_blocks,
                                     exp_runs, n_iters=n_iters,
                                     timing_mode=timing_mode, phases=phases)
    return _CACHE[key]


def _prep_inputs(x, attn_mask, W_qkv, b_qkv, W_out, b_out, gamma, beta,
                 chunk_at, tt_blocks):
    b_, s_, d_ = x.shape
    f32 = np.float32
    bf16 = ml_dtypes.bfloat16
    in_maps = []
    qsels = []
    bvrep = np.ascontiguousarray(
        np.broadcast_to(b_qkv[2 * d_:3 * d_][None, :], (128, d_))
    ).astype(bf16)
    wqkv_c = np.ascontiguousarray(W_qkv).astype(bf16)
    wout_c = np.ascontiguousarray(W_out).astype(bf16)
    nf = d_ // 128
    b_qkv = np.asarray(b_qkv, f32)
    vecs = ([b_qkv[f * 128:(f + 1) * 128] for f in range(nf)] +
            [b_qkv[d_ + f * 128:d_ + (f + 1) * 128] for f in range(nf)] +
            [np.asarray(b_out, f32)[f * 128:(f + 1) * 128] for f in range(nf)] +
            [np.asarray(gamma, f32)[f * 128:(f + 1) * 128] for f in range(nf)] +
            [np.asarray(beta, f32)[f * 128:(f + 1) * 128] for f in range(nf)])
    bpack = np.ascontiguousarray(np.stack(vecs, axis=1), dtype=f32)
    onesb = np.ones((128, 128), bf16)
    hd = d_ // H
    for core in range(N_CORES):
        bb, sh = core // 2, core % 2
        chunks = chunk_at[bb, sh]
        qsel = np.concatenate(
            [np.arange(c * QCHUNK, (c + 1) * QCHUNK) for c in chunks])
        qsels.append(qsel)
        xT_ = np.ascontiguousarray(x[bb].T).astype(bf16)
        xqT_ = np.ascontiguousarray(x[bb][qsel].T).astype(bf16)
        if tt_blocks:
            mt = np.stack([
                np.exp(np.ascontiguousarray(
                    attn_mask[bb,
                              chunks[p] * QCHUNK:(chunks[p] + 1) * QCHUNK,
                              t * KTILE:(t + 1) * KTILE].T,
                    dtype=np.float64) / np.sqrt(hd))
                for (p, t) in tt_blocks]).astype(bf16)
        else:
            mt = np.zeros((1, KTILE, QCHUNK), bf16)
        in_maps.append({
            "xT": xT_, "xqT": xqT_,
            "wqkv": wqkv_c, "wout": wout_c,
            "bpack": bpack,
            "bvrep": bvrep, "maskT": mt,
            "onesb": onesb,
        })
    return in_maps, qsels


def kernel(x, attn_mask, W_qkv, b_qkv, W_out, b_out, gamma, beta,
           n_iters=1):
    x = np.asarray(x, np.float32)
    attn_mask = np.asarray(attn_mask, np.float32)
    chunk_at, kt_lists, tt_blocks, exp_runs = _mask_pattern(attn_mask, B, S)
    pattern_key = (tuple(tuple(k) for k in kt_lists), tuple(tt_blocks),
                   tuple(sorted(exp_runs.items())))
    nc = _get_program(pattern_key, kt_lists, tt_blocks, exp_runs,
                      n_iters=n_iters)
    in_maps, qsels = _prep_inputs(
        x, attn_mask, np.asarray(W_qkv), np.asarray(b_qkv),
        np.asarray(W_out), np.asarray(b_out), np.asarray(gamma),
        np.asarray(beta), chunk_at, tt_blocks)
    res = run_bass_kernel_spmd(nc, in_maps, list(range(N_CORES)))
    out = np.empty((B, S, D), np.float32)
    for core in range(N_CORES):
        bb = core // 2
        out[bb, qsels[core]] = np.asarray(
            res.results[core]["yT"], dtype=np.float32).T
    return out
